# revision 28
# baseline (speedup 1.0000x reference)
"""Trainium2 Bass kernel for nn_DRA_40072044872030.

Key mathematical identity: in the reference, `_attention_module` applies
softmax over an axis of size 1, which is identically 1.0, so the module is
an exact identity map (wp = p * 1.0). The network therefore reduces to
`_composite_head(feature, ref_feature, ...)`:

    d = ref_feature - feature                         [B, 200, 56, 56]
    h = relu(BN(conv3x3(d, W) + cb))                  [B, 200, 56, 56]
    s = |conv1x1(h, w_s) + sb|                        [B, 56*56]
    out[b] = mean(top_313(s[b]))                      [B, 1]

Device implementation (8 NeuronCores, batch-sharded 2 images/core):
  - BN folded into conv weights/bias on host (weight preprocessing).
  - Images shipped in a zero-padded flat layout [margin | 58*58 | margin]
    per channel so the 3x3 conv becomes 9 shifted contiguous matmuls
    accumulated in PSUM; contraction ci -> 2x100 groups, outputs co ->
    2x100 groups.  d = ref - feat computed on device (in-place DVE sub).
  - float32r (full-rate fp32 matmul mode, ~12-bit mantissa) by default;
    exact fp32 matmul mode via PRECISION = "f32" (4x slower PE).
  - Exact top-k mean via GPSIMD kth_largest (exact 313th-largest value t),
    then mean = (sum(s where s > t) + (313 - count(s > t)) * t) / 313.
"""

import sys

if "/opt/trn_rl_repo" not in sys.path:
    sys.path.insert(0, "/opt/trn_rl_repo")

import numpy as np

import concourse.bass as bass
import concourse.tile as tile
from concourse import bacc, bass_isa, mybir
from concourse.bass_utils import run_bass_kernel_spmd

F32 = mybir.dt.float32
F32R = mybir.dt.float32r
F16 = mybir.dt.float16

N_CORES = 8
B = 16
C = 200
H = W = 56
HP = WP = 58                 # padded spatial
NPIX = H * W                 # 3136
NPAD = HP * WP               # 3364
MARGIN = 64                  # front margin of the padded flat buffer
PADLEN = MARGIN + NPAD + 60  # 3488 per-channel flat length
K_TOP = 313
BN_EPS = 1e-5
IMGS = B // N_CORES          # images per core
CG = 2                       # channel groups (ci and co), 100 each
GC = C // CG                 # 100
QT = 7                       # conv q-tiles, 8 rows each
QROWS = 8
QN = QROWS * WP              # 464 columns per conv matmul
SN = NPIX // QT              # 448 columns per s-matmul tile
PAD_N = 3200                 # kth_largest input size (128 * 25)
NEG = -1.0e30

PRECISION = "f32r"           # "f32r" (fast, ~1e-4 conv err) or "f32" (exact)


def _build_kernel(precision: str):
    nc = bacc.Bacc(None, target_bir_lowering=False)
    mmdt = F32R if precision == "f32r" else F32
    # inputs ship as fp16 in fast mode (halves DMA time; ~2^-12 rounding is
    # at the same scale as the f32r matmul rounding)
    idt = F16 if precision == "f32r" else F32

    feat_d = nc.dram_tensor("feat", [IMGS, C, PADLEN], idt, kind="ExternalInput")
    ref_d = nc.dram_tensor("ref", [IMGS, C, PADLEN], idt, kind="ExternalInput")
    # folded conv weights, laid out [ci_g, ci, (tap, og, co)]
    wl_d = nc.dram_tensor("wl", [CG, GC, 9 * CG * GC], idt, kind="ExternalInput")
    bias2_d = nc.dram_tensor("bias2", [GC, CG], F32, kind="ExternalInput")
    wsc_d = nc.dram_tensor("wsc", [GC, CG], F32, kind="ExternalInput")
    sb_d = nc.dram_tensor("sbias", [1, 1], F32, kind="ExternalInput")
    # topk consts: col r = (j+1)/128^(r+1) for threshold grids
    tkc_d = nc.dram_tensor("tkc", [128, 3], F32, kind="ExternalInput")
    ones_d = nc.dram_tensor("ones", [1, 128], F32, kind="ExternalInput")
    out_d = nc.dram_tensor("out", [IMGS, 1], F32, kind="ExternalOutput")
    nrounds = 2 if precision == "f32r" else 3

    import os
    _nonce = os.environ.get("KNONCE", "")
    with tile.TileContext(nc) as tc:
        with (
            tc.tile_pool(name=f"consts{_nonce}", bufs=1) as consts,
            tc.tile_pool(name="stage", bufs=2) as stage,
            tc.tile_pool(name="dpad", bufs=4) as dpad_pool,
            tc.tile_pool(name="hpool", bufs=3) as hpool,
            tc.tile_pool(name="spool", bufs=2) as spool,
            tc.tile_pool(name="small", bufs=2) as small,
            tc.tile_pool(name="cpsum", bufs=4, space="PSUM") as cpsum,
            tc.tile_pool(name="spsum", bufs=2, space="PSUM") as spsum,
            tc.tile_pool(name="bpsum", bufs=2, space="PSUM") as bpsum,
            tc.tile_pool(name="bcast", bufs=1) as bcast,
        ):
            # ---- constants (on the ACT HWDGE ring; img0 uses the SP ring) ----
            wl_in = consts.tile([GC, CG, 9 * CG * GC], idt)
            nc.scalar.dma_start(out=wl_in, in_=wl_d[:, :, :].rearrange("g c f -> c g f"))
            bias2 = consts.tile([GC, CG], F32)
            nc.scalar.dma_start(out=bias2, in_=bias2_d[:, :])
            wsc_f32 = consts.tile([GC, CG], F32)
            nc.scalar.dma_start(out=wsc_f32, in_=wsc_d[:, :])
            sbias = consts.tile([1, 1], F32)
            nc.scalar.dma_start(out=sbias, in_=sb_d[:, :])
            tkc = consts.tile([128, 3], F32)
            nc.scalar.dma_start(out=tkc, in_=tkc_d[:, :])
            ones_f32 = consts.tile([1, 128], F32)
            nc.scalar.dma_start(out=ones_f32, in_=ones_d[:, :])
            if precision == "f32r":
                wl = consts.tile([GC, CG, 9 * CG * GC], F32R)
                nc.vector.tensor_copy(wl, wl_in)
                wsc = consts.tile([GC, CG], F32R)
                nc.vector.tensor_copy(wsc, wsc_f32)
                ones_bc = consts.tile([1, 128], F32R)
                nc.vector.tensor_copy(ones_bc, ones_f32)
            else:
                wl, wsc, ones_bc = wl_in, wsc_f32, ones_f32
            out_sb = consts.tile([1, IMGS], F32)

            # rows 0..33 (covers conv q-tiles 0..3), then rows 34..57
            HALF0 = MARGIN + 34 * WP
            halves = [(0, HALF0), (HALF0, PADLEN)]
            for img in range(IMGS):
                # ---- d = ref - feat, in padded layout ----
                dma_eng = nc.sync if img == 0 else nc.scalar
                dpads = []
                pads = []
                for g in range(CG):
                    x_pad = stage.tile([GC, PADLEN], idt, tag="xpad")
                    r_pad = stage.tile([GC, PADLEN], idt, tag="rpad")
                    d_pad = dpad_pool.tile([GC, PADLEN], mmdt, tag="dpad")
                    pads.append((x_pad, r_pad, d_pad))
                    dpads.append(d_pad)
                # issue first halves of both groups before second halves
                for lo, hi in halves:
                    for g in range(CG):
                        x_pad, r_pad, d_pad = pads[g]
                        dma_eng.dma_start(
                            out=x_pad[:, lo:hi],
                            in_=feat_d[img, g * GC:(g + 1) * GC, lo:hi])
                        dma_eng.dma_start(
                            out=r_pad[:, lo:hi],
                            in_=ref_d[img, g * GC:(g + 1) * GC, lo:hi])
                    for g in range(CG):
                        x_pad, r_pad, d_pad = pads[g]
                        nc.vector.tensor_tensor(
                            out=d_pad[:, lo:hi], in0=r_pad[:, lo:hi],
                            in1=x_pad[:, lo:hi], op=mybir.AluOpType.subtract,
                        )

                # ---- conv 3x3 (+folded BN) + ReLU ----
                hs = []
                for og in range(CG):
                    h_t = hpool.tile([GC, NPIX], mmdt, tag="h")
                    hs.append(h_t)
                    for qt in range(QT):
                        ps = cpsum.tile([GC, QN], F32, tag="cps")
                        i = 0
                        for k in range(9):
                            ky, kx = divmod(k, 3)
                            off = (ky - 1) * WP + (kx - 1)
                            for g in range(CG):
                                base = MARGIN + WP + qt * QN + off
                                nc.tensor.matmul(
                                    ps,
                                    wl[:, g, (k * CG + og) * GC:(k * CG + og + 1) * GC],
                                    dpads[g][:, base:base + QN],
                                    start=(i == 0),
                                    stop=(i == 17),
                                )
                                i += 1
                        # BN+ReLU, keep only interior columns 1..56 per row
                        nc.scalar.activation(
                            out=h_t[:, qt * QROWS * W:(qt + 1) * QROWS * W]
                            .rearrange("p (r c) -> p r c", c=W),
                            in_=ps.rearrange("p (r c) -> p r c", c=WP)[:, :, 1:1 + W],
                            func=mybir.ActivationFunctionType.Relu,
                            bias=bias2[:, og:og + 1],
                            scale=1.0,
                        )

                # ---- s = |conv1x1(h) + sb| ----
                s32 = spool.tile([1, PAD_N], mmdt, tag="s32")
                nc.vector.memset(s32.bitcast(F32), NEG)
                for qt in range(QT):
                    sp = spsum.tile([1, SN], F32, tag="sps")
                    for og in range(CG):
                        nc.tensor.matmul(
                            sp,
                            wsc[:, og:og + 1],
                            hs[og][:, qt * SN:(qt + 1) * SN],
                            start=(og == 0),
                            stop=(og == 1),
                        )
                    nc.scalar.activation(
                        out=s32[:, qt * SN:(qt + 1) * SN],
                        in_=sp,
                        func=mybir.ActivationFunctionType.Abs,
                        bias=sbias,
                        scale=1.0,
                    )

                # ---- approximate 313th-largest threshold t (2-3 rounds of
                # 128-candidate counting; error in t is second-order in the
                # final mean), then exact count+sum against t ----
                s128 = small.tile([128, PAD_N // 128], F32, tag="s128")
                nc.sync.dma_start(out=s128, in_=s32.bitcast(F32))

                # replicate s to all partitions via PE (ones outer product);
                # track per-tile maxima as tiles land
                s_b = bcast.tile([128, NPIX], F32, tag="sb")
                mcols = small.tile([128, QT], F32, tag="mcols")
                for qt in range(QT):
                    bp = bpsum.tile([128, SN], F32, tag="bps")
                    nc.tensor.matmul(
                        bp, ones_bc, s32[0:1, qt * SN:(qt + 1) * SN],
                        start=True, stop=True,
                    )
                    nc.scalar.copy(
                        out=s_b[:, qt * SN:(qt + 1) * SN], in_=bp)
                    nc.vector.tensor_reduce(
                        out=mcols[:, qt:qt + 1],
                        in_=s_b[:, qt * SN:(qt + 1) * SN],
                        axis=mybir.AxisListType.X, op=mybir.AluOpType.max,
                    )

                # m = max(s), replicated on all partitions
                m_col = small.tile([128, 1], F32, tag="mcol")
                nc.vector.tensor_reduce(
                    out=m_col, in_=mcols, axis=mybir.AxisListType.X,
                    op=mybir.AluOpType.max,
                )
                mask = bcast.tile([128, NPIX], F32, tag="mask")
                cnt = small.tile([128, 1], F32, tag="cnt")
                g = small.tile([128, 1], F32, tag="g")
                sg = small.tile([128, 1], F32, tag="sg")
                tfin = small.tile([128, 1], F32, tag="tfin")
                tcand = small.tile([128, 1], F32, tag="tcand")
                u = small.tile([128, 1], F32, tag="u")
                nc.vector.memset(tfin, 0.0)
                for r in range(nrounds):
                    # candidates: tcand_j = tfin + m * (j+1)/128^(r+1)
                    nc.vector.tensor_tensor(
                        out=u, in0=m_col, in1=tkc[:, r:r + 1],
                        op=mybir.AluOpType.mult,
                    )
                    nc.vector.tensor_tensor(
                        out=tcand, in0=u, in1=tfin, op=mybir.AluOpType.add
                    )
                    nc.vector.tensor_scalar(
                        out=mask, in0=s_b, scalar1=tcand, scalar2=0.0,
                        op0=mybir.AluOpType.is_gt, op1=mybir.AluOpType.add,
                        accum_out=cnt,
                    )
                    nc.vector.tensor_scalar(
                        out=g, in0=cnt, scalar1=float(K_TOP), scalar2=None,
                        op0=mybir.AluOpType.is_ge,
                    )
                    nc.gpsimd.partition_all_reduce(
                        sg, g, channels=128, reduce_op=bass_isa.ReduceOp.add
                    )
                    # tfin += m * sg / 128^(r+1)
                    nc.vector.tensor_tensor(
                        out=u, in0=m_col, in1=sg, op=mybir.AluOpType.mult
                    )
                    nc.vector.scalar_tensor_tensor(
                        out=tfin, in0=u, scalar=1.0 / (128.0 ** (r + 1)),
                        in1=tfin, op0=mybir.AluOpType.mult,
                        op1=mybir.AluOpType.add,
                    )

                # exact count & masked sum of s against tfin
                mask25 = small.tile([128, PAD_N // 128], F32, tag="mask25")
                cs = small.tile([128, 2], F32, tag="cs")
                nc.vector.tensor_scalar(
                    out=mask25, in0=s128, scalar1=tfin, scalar2=None,
                    op0=mybir.AluOpType.is_gt,
                )
                nc.vector.tensor_reduce(
                    out=cs[:, 0:1], in_=mask25, axis=mybir.AxisListType.X,
                    op=mybir.AluOpType.add,
                )
                masked = small.tile([128, PAD_N // 128], F32, tag="masked")
                nc.vector.tensor_tensor(
                    out=masked, in0=mask25, in1=s128, op=mybir.AluOpType.mult
                )
                nc.vector.tensor_reduce(
                    out=cs[:, 1:2], in_=masked, axis=mybir.AxisListType.X,
                    op=mybir.AluOpType.add,
                )
                cs_red = small.tile([128, 2], F32, tag="csred")
                nc.gpsimd.partition_all_reduce(
                    cs_red, cs, channels=128, reduce_op=bass_isa.ReduceOp.add
                )
                tmp = small.tile([1, 1], F32, tag="tmp")
                # tmp = K_TOP - cnt
                nc.vector.tensor_scalar(
                    out=tmp, in0=cs_red[0:1, 0:1], scalar1=-1.0,
                    scalar2=float(K_TOP), op0=mybir.AluOpType.mult,
                    op1=mybir.AluOpType.add,
                )
                nc.vector.tensor_tensor(
                    out=tmp, in0=tmp, in1=tfin[0:1, 0:1], op=mybir.AluOpType.mult
                )
                nc.vector.tensor_tensor(
                    out=tmp, in0=tmp, in1=cs_red[0:1, 1:2], op=mybir.AluOpType.add
                )
                nc.vector.tensor_scalar(
                    out=out_sb[:, img:img + 1], in0=tmp, scalar1=1.0 / K_TOP,
                    scalar2=None, op0=mybir.AluOpType.mult,
                )

            nc.sync.dma_start(out=out_d[:, :], in_=out_sb)

    nc.compile()
    return nc


_KERNEL_CACHE = {}


def _get_kernel(precision):
    if precision not in _KERNEL_CACHE:
        _KERNEL_CACHE[precision] = _build_kernel(precision)
    return _KERNEL_CACHE[precision]


def _pad_images(a, dtype=np.float32):
    """[n, C, 56, 56] -> flat padded [n, C, PADLEN] with zero ring/margins."""
    n = a.shape[0]
    out = np.zeros((n, C, PADLEN), dtype)
    v = out[:, :, MARGIN:MARGIN + NPAD].reshape(n, C, HP, WP)
    v[:, :, 1:1 + H, 1:1 + W] = a
    return out


def _prepare_weights(c_w, c_b, bn_g, bn_b, bn_m, bn_v, score_w, score_b):
    scale = (bn_g / np.sqrt(bn_v + BN_EPS)).astype(np.float32)       # [co]
    wf = (c_w * scale[:, None, None, None]).astype(np.float32)       # [co,ci,3,3]
    bias2 = (scale * (c_b - bn_m) + bn_b).astype(np.float32)         # [co]

    # wl[g, ci, (k*CG+og)*GC + co] = wf[og*GC+co, g*GC+ci, ky, kx]
    w = wf.reshape(CG, GC, C, 3, 3)                  # [og, co, ci, ky, kx]
    w = w.transpose(2, 3, 4, 0, 1)                   # [ci, ky, kx, og, co]
    w = np.ascontiguousarray(w).reshape(CG, GC, 9 * CG * GC)
    wl = np.ascontiguousarray(w, dtype=np.float32)

    bias2_t = np.ascontiguousarray(bias2.reshape(CG, GC).T)          # [GC, og]
    wsc = np.ascontiguousarray(
        score_w.reshape(C).reshape(CG, GC).T.astype(np.float32))     # [GC, og]
    sb = np.array([[np.float32(np.asarray(score_b).reshape(-1)[0])]], np.float32)
    return wl, bias2_t, wsc, sb


def kernel(feature, ref_feature, c1_w, c1_b, c2_w, c2_b, fc1_w, fc1_b,
           fc2_w, fc2_b, comp_conv_w, comp_conv_b, bn_gamma, bn_beta,
           bn_mean, bn_var, score_w, score_b, _trace=False, _precision=None):
    feature = np.asarray(feature, np.float32)
    ref_feature = np.asarray(ref_feature, np.float32)
    wl, bias2, wsc, sb = _prepare_weights(
        np.asarray(comp_conv_w, np.float32), np.asarray(comp_conv_b, np.float32),
        np.asarray(bn_gamma, np.float32), np.asarray(bn_beta, np.float32),
        np.asarray(bn_mean, np.float32), np.asarray(bn_var, np.float32),
        np.asarray(score_w, np.float32), np.asarray(score_b, np.float32))

    precision_early = _precision or PRECISION
    in_np = np.float16 if precision_early == "f32r" else np.float32
    feat_pad = _pad_images(feature, in_np)
    ref_pad = _pad_images(ref_feature, in_np)
    j1 = np.arange(1, 129, dtype=np.float32)[:, None]
    tkc = np.concatenate([j1 / 128.0 ** (r + 1) for r in range(3)], axis=1)
    tkc = np.ascontiguousarray(tkc, np.float32)

    precision = precision_early
    nc = _get_kernel(precision)
    wl_in = wl.astype(in_np)
    in_maps = []
    for r in range(N_CORES):
        sl = slice(r * IMGS, (r + 1) * IMGS)
        in_maps.append(dict(
            feat=np.ascontiguousarray(feat_pad[sl]),
            ref=np.ascontiguousarray(ref_pad[sl]),
            wl=wl_in, bias2=bias2, wsc=wsc, sbias=sb, tkc=tkc,
            ones=np.ones((1, 128), np.float32),
        ))
    res = run_bass_kernel_spmd(
        nc, in_maps, core_ids=list(range(N_CORES)), trace=_trace
    )
    out = np.concatenate([res.results[r]["out"] for r in range(N_CORES)], axis=0)
    if _trace:
        kernel.last_exec_time_ns = res.exec_time_ns
        kernel.last_results = res
    return out.astype(np.float32)


# revision 30
# speedup vs baseline: 1.1694x; 1.1694x over previous
"""Trainium2 Bass kernel for nn_DRA_40072044872030.

Key mathematical identity: in the reference, `_attention_module` applies
softmax over an axis of size 1, which is identically 1.0, so the module is
an exact identity map (wp = p * 1.0). The network therefore reduces to
`_composite_head(feature, ref_feature, ...)`:

    d = ref_feature - feature                         [B, 200, 56, 56]
    h = relu(BN(conv3x3(d, W) + cb))                  [B, 200, 56, 56]
    s = |conv1x1(h, w_s) + sb|                        [B, 56*56]
    out[b] = mean(top_313(s[b]))                      [B, 1]

Device implementation (8 NeuronCores, batch-sharded 2 images/core):
  - BN folded into conv weights/bias on host (weight preprocessing).
  - Images shipped in a zero-padded flat layout [margin | 58*58 | margin]
    per channel so the 3x3 conv becomes 9 shifted contiguous matmuls
    accumulated in PSUM; contraction ci -> 2x100 groups, outputs co ->
    2x100 groups.  d = ref - feat computed on device (in-place DVE sub).
  - float32r (full-rate fp32 matmul mode, ~12-bit mantissa) by default;
    exact fp32 matmul mode via PRECISION = "f32" (4x slower PE).
  - Exact top-k mean via GPSIMD kth_largest (exact 313th-largest value t),
    then mean = (sum(s where s > t) + (313 - count(s > t)) * t) / 313.
"""

import sys

if "/opt/trn_rl_repo" not in sys.path:
    sys.path.insert(0, "/opt/trn_rl_repo")

import numpy as np

import concourse.bass as bass
import concourse.tile as tile
from concourse import bacc, bass_isa, mybir
from concourse.bass_utils import run_bass_kernel_spmd

F32 = mybir.dt.float32
F32R = mybir.dt.float32r
F16 = mybir.dt.float16

N_CORES = 8
B = 16
C = 200
H = W = 56
HP = WP = 58                 # padded spatial
NPIX = H * W                 # 3136
NPAD = HP * WP               # 3364
MARGIN = 64                  # front margin of the padded flat buffer
PADLEN = MARGIN + NPAD + 60  # 3488 per-channel flat length
K_TOP = 313
BN_EPS = 1e-5
IMGS = B // N_CORES          # images per core
CG = 2                       # channel groups (ci and co), 100 each
GC = C // CG                 # 100
QT = 7                       # conv q-tiles, 8 rows each
QROWS = 8
QN = QROWS * WP              # 464 columns per conv matmul
SN = NPIX // QT              # 448 columns per s-matmul tile
PAD_N = 3200                 # kth_largest input size (128 * 25)
NEG = -1.0e30

PRECISION = "f32r"           # "f32r" (fast, ~1e-4 conv err) or "f32" (exact)


def _build_kernel(precision: str):
    nc = bacc.Bacc(None, target_bir_lowering=False)
    mmdt = F32R if precision == "f32r" else F32
    # inputs ship as fp16 in fast mode (halves DMA time; ~2^-12 rounding is
    # at the same scale as the f32r matmul rounding)
    idt = F16 if precision == "f32r" else F32

    feat_d = nc.dram_tensor("feat", [IMGS, C, PADLEN], idt, kind="ExternalInput")
    ref_d = nc.dram_tensor("ref", [IMGS, C, PADLEN], idt, kind="ExternalInput")
    # folded conv weights, laid out [ci_g, ci, (tap, og, co)]
    wl_d = nc.dram_tensor("wl", [CG, GC, 9 * CG * GC], idt, kind="ExternalInput")
    bias2_d = nc.dram_tensor("bias2", [GC, CG], F32, kind="ExternalInput")
    wsc_d = nc.dram_tensor("wsc", [GC, CG], F32, kind="ExternalInput")
    sb_d = nc.dram_tensor("sbias", [1, 1], F32, kind="ExternalInput")
    # topk consts: col r = (j+1)/128^(r+1) for threshold grids
    tkc_d = nc.dram_tensor("tkc", [128, 3], F32, kind="ExternalInput")
    ones_d = nc.dram_tensor("ones", [1, 128], F32, kind="ExternalInput")
    out_d = nc.dram_tensor("out", [IMGS, 1], F32, kind="ExternalOutput")
    nrounds = 2 if precision == "f32r" else 3

    import os
    _nonce = os.environ.get("KNONCE", "")
    with tile.TileContext(nc) as tc:
        with (
            tc.tile_pool(name=f"consts{_nonce}", bufs=1) as consts,
            tc.tile_pool(name="stage", bufs=2) as stage,
            tc.tile_pool(name="dpad", bufs=4) as dpad_pool,
            tc.tile_pool(name="hpool", bufs=3) as hpool,
            tc.tile_pool(name="spool", bufs=2) as spool,
            tc.tile_pool(name="small", bufs=2) as small,
            tc.tile_pool(name="cpsum", bufs=4, space="PSUM") as cpsum,
            tc.tile_pool(name="spsum", bufs=2, space="PSUM") as spsum,
            tc.tile_pool(name="bpsum", bufs=2, space="PSUM") as bpsum,
            tc.tile_pool(name="bcast", bufs=1) as bcast,
        ):
            # ---- constants (on the ACT HWDGE ring; img0 uses the SP ring) ----
            wl_in = consts.tile([GC, CG, 9 * CG * GC], idt)
            nc.scalar.dma_start(out=wl_in, in_=wl_d[:, :, :].rearrange("g c f -> c g f"))
            bias2 = consts.tile([GC, CG], F32)
            nc.scalar.dma_start(out=bias2, in_=bias2_d[:, :])
            wsc_f32 = consts.tile([GC, CG], F32)
            nc.scalar.dma_start(out=wsc_f32, in_=wsc_d[:, :])
            sbias = consts.tile([1, 1], F32)
            nc.scalar.dma_start(out=sbias, in_=sb_d[:, :])
            tkc = consts.tile([128, 3], F32)
            nc.scalar.dma_start(out=tkc, in_=tkc_d[:, :])
            ones_f32 = consts.tile([1, 128], F32)
            nc.scalar.dma_start(out=ones_f32, in_=ones_d[:, :])
            if precision == "f32r":
                wl = consts.tile([GC, CG, 9 * CG * GC], F32R)
                nc.vector.tensor_copy(wl, wl_in)
                wsc = consts.tile([GC, CG], F32R)
                nc.vector.tensor_copy(wsc, wsc_f32)
                ones_bc = consts.tile([1, 128], F32R)
                nc.vector.tensor_copy(ones_bc, ones_f32)
            else:
                wl, wsc, ones_bc = wl_in, wsc_f32, ones_f32
            out_sb = consts.tile([1, IMGS], F32)

            # rows 0..33 (covers conv q-tiles 0..3), then rows 34..57
            HALF0 = MARGIN + 34 * WP
            halves = [(0, HALF0), (HALF0, PADLEN)]
            for img in range(IMGS):
                # ---- d = ref - feat, in padded layout ----
                dma_eng = nc.sync if img == 0 else nc.scalar
                dpads = []
                pads = []
                for g in range(CG):
                    x_pad = stage.tile([GC, PADLEN], idt, tag="xpad")
                    r_pad = stage.tile([GC, PADLEN], idt, tag="rpad")
                    d_pad = dpad_pool.tile([GC, PADLEN], mmdt, tag="dpad")
                    pads.append((x_pad, r_pad, d_pad))
                    dpads.append(d_pad)
                # issue first halves of both groups before second halves
                for lo, hi in halves:
                    for g in range(CG):
                        x_pad, r_pad, d_pad = pads[g]
                        dma_eng.dma_start(
                            out=x_pad[:, lo:hi],
                            in_=feat_d[img, g * GC:(g + 1) * GC, lo:hi])
                        dma_eng.dma_start(
                            out=r_pad[:, lo:hi],
                            in_=ref_d[img, g * GC:(g + 1) * GC, lo:hi])
                    for g in range(CG):
                        x_pad, r_pad, d_pad = pads[g]
                        nc.vector.tensor_tensor(
                            out=d_pad[:, lo:hi], in0=r_pad[:, lo:hi],
                            in1=x_pad[:, lo:hi], op=mybir.AluOpType.subtract,
                        )

                # ---- conv 3x3 (+folded BN) + ReLU ----
                # qt innermost (groups of 4) so consecutive matmuls share the
                # stationary operand and LDWEIGHTS can be elided/overlapped
                hs = []
                for og in range(CG):
                    h_t = hpool.tile([GC, NPIX], mmdt, tag="h")
                    hs.append(h_t)
                    for qt0 in range(0, QT, 4):
                        qts = range(qt0, min(qt0 + 4, QT))
                        pss = {qt: cpsum.tile([GC, QN], F32, tag="cps",
                                              name=f"cps_{img}_{og}_{qt}")
                               for qt in qts}
                        for g in range(CG):
                            for k in range(9):
                                ky, kx = divmod(k, 3)
                                off = (ky - 1) * WP + (kx - 1)
                                wslice = wl[:, g, (k * CG + og) * GC:
                                            (k * CG + og + 1) * GC]
                                for qt in qts:
                                    base = MARGIN + WP + qt * QN + off
                                    nc.tensor.matmul(
                                        pss[qt], wslice,
                                        dpads[g][:, base:base + QN],
                                        start=(g == 0 and k == 0),
                                        stop=(g == 1 and k == 8),
                                    )
                        for qt in qts:
                            # BN+ReLU, keep interior columns 1..56 per row
                            nc.scalar.activation(
                                out=h_t[:, qt * QROWS * W:(qt + 1) * QROWS * W]
                                .rearrange("p (r c) -> p r c", c=W),
                                in_=pss[qt].rearrange(
                                    "p (r c) -> p r c", c=WP)[:, :, 1:1 + W],
                                func=mybir.ActivationFunctionType.Relu,
                                bias=bias2[:, og:og + 1],
                                scale=1.0,
                            )

                # ---- s = |conv1x1(h) + sb| ----
                s32 = spool.tile([1, PAD_N], mmdt, tag="s32")
                nc.vector.memset(s32.bitcast(F32), NEG)
                for qt in range(QT):
                    sp = spsum.tile([1, SN], F32, tag="sps")
                    for og in range(CG):
                        nc.tensor.matmul(
                            sp,
                            wsc[:, og:og + 1],
                            hs[og][:, qt * SN:(qt + 1) * SN],
                            start=(og == 0),
                            stop=(og == 1),
                        )
                    nc.scalar.activation(
                        out=s32[:, qt * SN:(qt + 1) * SN],
                        in_=sp,
                        func=mybir.ActivationFunctionType.Abs,
                        bias=sbias,
                        scale=1.0,
                    )

                # ---- approximate 313th-largest threshold t (2-3 rounds of
                # 128-candidate counting; error in t is second-order in the
                # final mean), then exact count+sum against t ----
                s128 = small.tile([128, PAD_N // 128], F32, tag="s128")
                nc.sync.dma_start(out=s128, in_=s32.bitcast(F32))

                # replicate s to all partitions via PE (ones outer product);
                # track per-tile maxima as tiles land
                s_b = bcast.tile([128, NPIX], F32, tag="sb")
                mcols = small.tile([128, QT], F32, tag="mcols")
                for qt in range(QT):
                    bp = bpsum.tile([128, SN], F32, tag="bps")
                    nc.tensor.matmul(
                        bp, ones_bc, s32[0:1, qt * SN:(qt + 1) * SN],
                        start=True, stop=True,
                    )
                    nc.scalar.copy(
                        out=s_b[:, qt * SN:(qt + 1) * SN], in_=bp)
                    nc.vector.tensor_reduce(
                        out=mcols[:, qt:qt + 1],
                        in_=s_b[:, qt * SN:(qt + 1) * SN],
                        axis=mybir.AxisListType.X, op=mybir.AluOpType.max,
                    )

                # m = max(s), replicated on all partitions
                m_col = small.tile([128, 1], F32, tag="mcol")
                nc.vector.tensor_reduce(
                    out=m_col, in_=mcols, axis=mybir.AxisListType.X,
                    op=mybir.AluOpType.max,
                )
                mask = bcast.tile([128, NPIX], F32, tag="mask")
                cnt = small.tile([128, 1], F32, tag="cnt")
                g = small.tile([128, 1], F32, tag="g")
                sg = small.tile([128, 1], F32, tag="sg")
                tfin = small.tile([128, 1], F32, tag="tfin")
                tcand = small.tile([128, 1], F32, tag="tcand")
                u = small.tile([128, 1], F32, tag="u")
                nc.vector.memset(tfin, 0.0)
                for r in range(nrounds):
                    # candidates: tcand_j = tfin + m * (j+1)/128^(r+1)
                    nc.vector.tensor_tensor(
                        out=u, in0=m_col, in1=tkc[:, r:r + 1],
                        op=mybir.AluOpType.mult,
                    )
                    nc.vector.tensor_tensor(
                        out=tcand, in0=u, in1=tfin, op=mybir.AluOpType.add
                    )
                    nc.vector.tensor_scalar(
                        out=mask, in0=s_b, scalar1=tcand, scalar2=0.0,
                        op0=mybir.AluOpType.is_gt, op1=mybir.AluOpType.add,
                        accum_out=cnt,
                    )
                    nc.vector.tensor_scalar(
                        out=g, in0=cnt, scalar1=float(K_TOP), scalar2=None,
                        op0=mybir.AluOpType.is_ge,
                    )
                    nc.gpsimd.partition_all_reduce(
                        sg, g, channels=128, reduce_op=bass_isa.ReduceOp.add
                    )
                    # tfin += m * sg / 128^(r+1)
                    nc.vector.tensor_tensor(
                        out=u, in0=m_col, in1=sg, op=mybir.AluOpType.mult
                    )
                    nc.vector.scalar_tensor_tensor(
                        out=tfin, in0=u, scalar=1.0 / (128.0 ** (r + 1)),
                        in1=tfin, op0=mybir.AluOpType.mult,
                        op1=mybir.AluOpType.add,
                    )

                # exact count & masked sum of s against tfin
                mask25 = small.tile([128, PAD_N // 128], F32, tag="mask25")
                cs = small.tile([128, 2], F32, tag="cs")
                nc.vector.tensor_scalar(
                    out=mask25, in0=s128, scalar1=tfin, scalar2=None,
                    op0=mybir.AluOpType.is_gt,
                )
                nc.vector.tensor_reduce(
                    out=cs[:, 0:1], in_=mask25, axis=mybir.AxisListType.X,
                    op=mybir.AluOpType.add,
                )
                masked = small.tile([128, PAD_N // 128], F32, tag="masked")
                nc.vector.tensor_tensor(
                    out=masked, in0=mask25, in1=s128, op=mybir.AluOpType.mult
                )
                nc.vector.tensor_reduce(
                    out=cs[:, 1:2], in_=masked, axis=mybir.AxisListType.X,
                    op=mybir.AluOpType.add,
                )
                cs_red = small.tile([128, 2], F32, tag="csred")
                nc.gpsimd.partition_all_reduce(
                    cs_red, cs, channels=128, reduce_op=bass_isa.ReduceOp.add
                )
                tmp = small.tile([1, 1], F32, tag="tmp")
                # tmp = K_TOP - cnt
                nc.vector.tensor_scalar(
                    out=tmp, in0=cs_red[0:1, 0:1], scalar1=-1.0,
                    scalar2=float(K_TOP), op0=mybir.AluOpType.mult,
                    op1=mybir.AluOpType.add,
                )
                nc.vector.tensor_tensor(
                    out=tmp, in0=tmp, in1=tfin[0:1, 0:1], op=mybir.AluOpType.mult
                )
                nc.vector.tensor_tensor(
                    out=tmp, in0=tmp, in1=cs_red[0:1, 1:2], op=mybir.AluOpType.add
                )
                nc.vector.tensor_scalar(
                    out=out_sb[:, img:img + 1], in0=tmp, scalar1=1.0 / K_TOP,
                    scalar2=None, op0=mybir.AluOpType.mult,
                )

            nc.sync.dma_start(out=out_d[:, :], in_=out_sb)

    nc.compile()
    return nc


_KERNEL_CACHE = {}


def _get_kernel(precision):
    if precision not in _KERNEL_CACHE:
        _KERNEL_CACHE[precision] = _build_kernel(precision)
    return _KERNEL_CACHE[precision]


def _pad_images(a, dtype=np.float32):
    """[n, C, 56, 56] -> flat padded [n, C, PADLEN] with zero ring/margins."""
    n = a.shape[0]
    out = np.zeros((n, C, PADLEN), dtype)
    v = out[:, :, MARGIN:MARGIN + NPAD].reshape(n, C, HP, WP)
    v[:, :, 1:1 + H, 1:1 + W] = a
    return out


def _prepare_weights(c_w, c_b, bn_g, bn_b, bn_m, bn_v, score_w, score_b):
    scale = (bn_g / np.sqrt(bn_v + BN_EPS)).astype(np.float32)       # [co]
    wf = (c_w * scale[:, None, None, None]).astype(np.float32)       # [co,ci,3,3]
    bias2 = (scale * (c_b - bn_m) + bn_b).astype(np.float32)         # [co]

    # wl[g, ci, (k*CG+og)*GC + co] = wf[og*GC+co, g*GC+ci, ky, kx]
    w = wf.reshape(CG, GC, C, 3, 3)                  # [og, co, ci, ky, kx]
    w = w.transpose(2, 3, 4, 0, 1)                   # [ci, ky, kx, og, co]
    w = np.ascontiguousarray(w).reshape(CG, GC, 9 * CG * GC)
    wl = np.ascontiguousarray(w, dtype=np.float32)

    bias2_t = np.ascontiguousarray(bias2.reshape(CG, GC).T)          # [GC, og]
    wsc = np.ascontiguousarray(
        score_w.reshape(C).reshape(CG, GC).T.astype(np.float32))     # [GC, og]
    sb = np.array([[np.float32(np.asarray(score_b).reshape(-1)[0])]], np.float32)
    return wl, bias2_t, wsc, sb


def kernel(feature, ref_feature, c1_w, c1_b, c2_w, c2_b, fc1_w, fc1_b,
           fc2_w, fc2_b, comp_conv_w, comp_conv_b, bn_gamma, bn_beta,
           bn_mean, bn_var, score_w, score_b, _trace=False, _precision=None):
    feature = np.asarray(feature, np.float32)
    ref_feature = np.asarray(ref_feature, np.float32)
    wl, bias2, wsc, sb = _prepare_weights(
        np.asarray(comp_conv_w, np.float32), np.asarray(comp_conv_b, np.float32),
        np.asarray(bn_gamma, np.float32), np.asarray(bn_beta, np.float32),
        np.asarray(bn_mean, np.float32), np.asarray(bn_var, np.float32),
        np.asarray(score_w, np.float32), np.asarray(score_b, np.float32))

    precision_early = _precision or PRECISION
    in_np = np.float16 if precision_early == "f32r" else np.float32
    feat_pad = _pad_images(feature, in_np)
    ref_pad = _pad_images(ref_feature, in_np)
    j1 = np.arange(1, 129, dtype=np.float32)[:, None]
    tkc = np.concatenate([j1 / 128.0 ** (r + 1) for r in range(3)], axis=1)
    tkc = np.ascontiguousarray(tkc, np.float32)

    precision = precision_early
    nc = _get_kernel(precision)
    wl_in = wl.astype(in_np)
    in_maps = []
    for r in range(N_CORES):
        sl = slice(r * IMGS, (r + 1) * IMGS)
        in_maps.append(dict(
            feat=np.ascontiguousarray(feat_pad[sl]),
            ref=np.ascontiguousarray(ref_pad[sl]),
            wl=wl_in, bias2=bias2, wsc=wsc, sbias=sb, tkc=tkc,
            ones=np.ones((1, 128), np.float32),
        ))
    res = run_bass_kernel_spmd(
        nc, in_maps, core_ids=list(range(N_CORES)), trace=_trace
    )
    out = np.concatenate([res.results[r]["out"] for r in range(N_CORES)], axis=0)
    if _trace:
        kernel.last_exec_time_ns = res.exec_time_ns
        kernel.last_results = res
    return out.astype(np.float32)
